# revision 7
# baseline (speedup 1.0000x reference)
"""Trainium2 Bass kernel for gated pair-bias attention (B=8,S=1024,D=256,H=8,DH=32).

Sharding: data-parallel over batch — core b computes batch element b entirely.
Weights + pair bias replicated to all 8 cores.

Per-core math (batch element dropped):
  g     = sigmoid(q @ WgT + bg)                       [S, E]
  qh    = (q @ WqT) * DH^-0.5 ; kh = k @ WkT ; vh = v @ WvT
  s_hqk = qh_h @ kh_h^T + mask + bias_h               (mask handled host-side)
  attn  = softmax_k(s) ;  o = attn @ vh_h ;  o = g * o ;  out = o @ WoT

Kernel strategy (all f32):
  - everything transposed on-chip via PE+identity matmuls (inputs d-major,
    weights d-major) so every contraction has its axis on partitions
  - scores computed TRANSPOSED: sT[k, q] = khT_h^T-slice @ qhT_h  (K=dh=32)
  - pair bias added by PE transpose-accumulate: matmul(psum_sT, lhsT=bias_blk,
    rhs=I, start=False) adds bias[h,q,k]^T into the scores psum for free on DVE
  - exp on ACT straight PSUM->SBUF (no max subtraction: |scores| <= ~7, exact
    in f32), flash-style: unnormalized oT = vh^T-slice @ expT accumulated over
    k tiles, row sums via an all-ones stationary in a second col-group, divide
    once at the end ([S] elems, not [S,S])
"""

import os
import sys

import numpy as np

for _p in ("/opt/trn_rl_repo", "/root/.axon_site/_ro/trn_rl_repo"):
    if os.path.isdir(_p) and _p not in sys.path:
        sys.path.append(_p)

import concourse.bass as bass
import concourse.mybir as mybir
import concourse.tile as tile
from concourse import bacc
from concourse.bass_utils import run_bass_kernel_spmd
from concourse.masks import make_identity

S, D, E, H, DH = 1024, 256, 256, 8, 32
NCORES = 8
F32 = mybir.dt.float32
NORM = float(DH) ** -0.5
ST = S // 128   # 8 s-tiles
DT = D // 128   # 2 d-tiles
ET = E // 128   # 2 e-tiles
Act = mybir.ActivationFunctionType


def build_bass() -> bass.Bass:
    # Bacc (not raw Bass): its compile() runs move_matmul_waits_to_ldweights +
    # generate_event_semaphores, which split multi-semaphore waits that the
    # TRN2 instruction encodings cannot carry (walrus rejects them otherwise).
    nc = bacc.Bacc("TRN2", target_bir_lowering=False, debug=True)

    q_d = nc.dram_tensor("q", [S, D], F32, kind="ExternalInput")
    k_d = nc.dram_tensor("k", [S, D], F32, kind="ExternalInput")
    v_d = nc.dram_tensor("v", [S, D], F32, kind="ExternalInput")
    bias_d = nc.dram_tensor("bias", [H, S, S], F32, kind="ExternalInput")
    w_d = {
        "q": nc.dram_tensor("Wq", [E, D], F32, kind="ExternalInput"),
        "k": nc.dram_tensor("Wk", [E, D], F32, kind="ExternalInput"),
        "v": nc.dram_tensor("Wv", [E, D], F32, kind="ExternalInput"),
        "g": nc.dram_tensor("Wg", [E, D], F32, kind="ExternalInput"),
        "o": nc.dram_tensor("Wo", [D, E], F32, kind="ExternalInput"),
    }
    bg_d = nc.dram_tensor("bg", [E], F32, kind="ExternalInput")
    out_d = nc.dram_tensor("out", [S, D], F32, kind="ExternalOutput")

    with tile.TileContext(nc) as tc:
        with (
            tc.tile_pool(name="const", bufs=1) as constp,
            tc.tile_pool(name="wpool", bufs=1) as wpool,
            tc.tile_pool(name="persist", bufs=1) as persist,
            tc.tile_pool(name="nat", bufs=3) as nat,
            tc.tile_pool(name="biasp", bufs=12) as biasp,
            tc.tile_pool(name="expp", bufs=3) as expp,
            tc.tile_pool(name="smallp", bufs=4) as smallp,
            tc.tile_pool(name="outp", bufs=3) as outp,
            tc.tile_pool(name="psum", bufs=2, space="PSUM") as psum,
        ):
            ident = constp.tile([128, 128], F32)
            make_identity(nc, ident[:])
            ones_l = constp.tile([128, DH], F32)
            nc.gpsimd.memset(ones_l[:], 1.0)
            bg_sb = constp.tile([128, ET], F32)
            bg2d = bg_d.rearrange("(a b) -> a b", b=1)
            for et in range(ET):
                nc.sync.dma_start(out=bg_sb[:, et : et + 1],
                                  in_=bg2d[et * 128 : (et + 1) * 128, :])

            # ---- transposed weights: WT[name][dt] = W^T[d-tile, e(256)] ----
            WT = {}
            for nm, wd in w_d.items():
                wts = [wpool.tile([128, E], F32, name=f"WT_{nm}{i}",
                                  tag=f"WT_{nm}{i}") for i in range(2)]
                for rt in range(2):  # row tile of natural W
                    wnat = nat.tile([128, 256], F32, tag="wnat")
                    nc.sync.dma_start(out=wnat[:], in_=wd[rt * 128 : (rt + 1) * 128, :])
                    for ct in range(2):  # col tile = partition tile of W^T
                        ps_tr = psum.tile([128, 128], F32, tag="ps_big", bufs=3)
                        # regular matmul (not is_transpose): out = wnat_blk^T @ I.
                        # The pure-LDWEIGHTS transpose path only admits one
                        # sync wait and walrus rejects it here.
                        nc.tensor.matmul(ps_tr[:], lhsT=wnat[:, ct * 128 : (ct + 1) * 128],
                                         rhs=ident[:], start=True, stop=True)
                        dst = wts[ct][:, rt * 128 : (rt + 1) * 128]
                        if nm == "q":  # fold the 1/sqrt(DH) score scale into WqT
                            nc.vector.tensor_scalar_mul(dst, ps_tr[:], NORM)
                        else:
                            nc.vector.tensor_copy(dst, ps_tr[:])
                WT[nm] = wts

            # ---- transposed inputs: xT[dt] = x^T[d-tile, s(1024)] ----
            def load_transposed(src_d, pref):
                tiles = [persist.tile([128, S], F32, name=f"{pref}T{i}",
                                      tag=f"inT{i}", bufs=2) for i in range(DT)]
                for st in range(ST):
                    xnat = nat.tile([128, D], F32, tag="xnat")
                    nc.sync.dma_start(out=xnat[:], in_=src_d[st * 128 : (st + 1) * 128, :])
                    for dt in range(DT):
                        ps_tr = psum.tile([128, 128], F32, tag="ps_big", bufs=3)
                        nc.tensor.matmul(ps_tr[:], lhsT=xnat[:, dt * 128 : (dt + 1) * 128],
                                         rhs=ident[:], start=True, stop=True)
                        nc.vector.tensor_copy(tiles[dt][:, st * 128 : (st + 1) * 128],
                                              ps_tr[:])
                return tiles

            # ---- projections ----
            qT = load_transposed(q_d, "q")
            qhT = [persist.tile([128, S], F32, name=f"qhT{i}") for i in range(ET)]
            gateT = [persist.tile([128, S], F32, name=f"gateT{i}") for i in range(ET)]
            for et in range(ET):
                ps_p = psum.tile([128, S], F32, tag="ps_big", bufs=3)
                for dt in range(DT):
                    for qc in range(2):
                        nc.tensor.matmul(
                            ps_p[:, qc * 512 : (qc + 1) * 512],
                            lhsT=WT["q"][dt][:, et * 128 : (et + 1) * 128],
                            rhs=qT[dt][:, qc * 512 : (qc + 1) * 512],
                            start=(dt == 0), stop=(dt == DT - 1))
                nc.vector.tensor_copy(qhT[et][:], ps_p[:])
                ps_g = psum.tile([128, S], F32, tag="ps_big", bufs=3)
                for dt in range(DT):
                    for qc in range(2):
                        nc.tensor.matmul(
                            ps_g[:, qc * 512 : (qc + 1) * 512],
                            lhsT=WT["g"][dt][:, et * 128 : (et + 1) * 128],
                            rhs=qT[dt][:, qc * 512 : (qc + 1) * 512],
                            start=(dt == 0), stop=(dt == DT - 1))
                nc.scalar.activation(gateT[et][:], ps_g[:], Act.Sigmoid,
                                     bias=bg_sb[:, et : et + 1])

            kT = load_transposed(k_d, "k")
            khT = [persist.tile([128, S], F32, name=f"khT{i}") for i in range(ET)]
            for et in range(ET):
                ps_p = psum.tile([128, S], F32, tag="ps_big", bufs=3)
                for dt in range(DT):
                    for qc in range(2):
                        nc.tensor.matmul(
                            ps_p[:, qc * 512 : (qc + 1) * 512],
                            lhsT=WT["k"][dt][:, et * 128 : (et + 1) * 128],
                            rhs=kT[dt][:, qc * 512 : (qc + 1) * 512],
                            start=(dt == 0), stop=(dt == DT - 1))
                nc.vector.tensor_copy(khT[et][:], ps_p[:])

            vT = load_transposed(v_d, "v")
            vh = [persist.tile([128, E], F32, name=f"vh{i}") for i in range(ST)]
            for st in range(ST):
                ps_v = psum.tile([128, E], F32, tag="ps_big", bufs=3)
                for dt in range(DT):
                    nc.tensor.matmul(ps_v[:],
                                     lhsT=vT[dt][:, st * 128 : (st + 1) * 128],
                                     rhs=WT["v"][dt][:],
                                     start=(dt == 0), stop=(dt == DT - 1))
                nc.vector.tensor_copy(vh[st][:], ps_v[:])

            # ---- attention, one head at a time ----
            o_gT = [persist.tile([128, S], F32, name=f"o_gT{i}") for i in range(ET)]
            for h in range(H):
                et, hr = h // 4, (h % 4) * DH
                slabs = []
                for qb in range(ST):
                    bslab = biasp.tile([128, S], F32, tag="bslab",
                                       name=f"bslab_h{h}_q{qb}")
                    nc.sync.dma_start(out=bslab[:],
                                      in_=bias_d[h, qb * 128 : (qb + 1) * 128, :])
                    slabs.append(bslab)
                ps_o = psum.tile([64, S], F32, tag="ps_o", bufs=1)
                for kt in range(ST):
                    ps_s = psum.tile([128, S], F32, tag="ps_big", bufs=3)
                    for qc in range(2):  # sT[k_tile, q] = khT_h-slice^T @ qhT_h
                        nc.tensor.matmul(
                            ps_s[:, qc * 512 : (qc + 1) * 512],
                            lhsT=khT[et][hr : hr + DH, kt * 128 : (kt + 1) * 128],
                            rhs=qhT[et][hr : hr + DH, qc * 512 : (qc + 1) * 512],
                            start=True, stop=False,
                            tile_position=(hr, 0))
                    for qb in range(ST):  # += bias[h, q_blk, k_tile]^T
                        nc.tensor.matmul(
                            ps_s[:, qb * 128 : (qb + 1) * 128],
                            lhsT=slabs[qb][:, kt * 128 : (kt + 1) * 128],
                            rhs=ident[:],
                            start=False, stop=(qb % 4 == 3))
                    expT = expp.tile([128, S], F32, tag="expT")
                    nc.scalar.activation(expT[:], ps_s[:], Act.Exp)
                    for qc in range(2):  # oT & sigma accumulate over k tiles
                        qcs = slice(qc * 512, (qc + 1) * 512)
                        nc.tensor.matmul(ps_o[0:DH, qcs],
                                         lhsT=vh[kt][:, h * DH : (h + 1) * DH],
                                         rhs=expT[:, qcs],
                                         start=(kt == 0), stop=(kt == ST - 1),
                                         tile_position=(0, 0))
                        nc.tensor.matmul(ps_o[DH : 2 * DH, qcs],
                                         lhsT=ones_l[:],
                                         rhs=expT[:, qcs],
                                         start=(kt == 0), stop=(kt == ST - 1),
                                         tile_position=(0, DH))
                # normalize + gate:  o_gT[h rows] = oT * gateT * (1/sigma)
                rsig = smallp.tile([DH, S], F32, tag="rsig")
                nc.vector.reciprocal(rsig[:], ps_o[DH : 2 * DH, :])
                tmp_o = smallp.tile([DH, S], F32, tag="tmp_o")
                nc.vector.tensor_mul(tmp_o[:], ps_o[0:DH, :], gateT[et][hr : hr + DH, :])
                nc.vector.tensor_mul(o_gT[et][hr : hr + DH, :], tmp_o[:], rsig[:])

            # ---- output projection ----
            for st in range(ST):
                ps_out = psum.tile([128, D], F32, tag="ps_big", bufs=3)
                for et in range(ET):
                    nc.tensor.matmul(ps_out[:],
                                     lhsT=o_gT[et][:, st * 128 : (st + 1) * 128],
                                     rhs=WT["o"][et][:],
                                     start=(et == 0), stop=(et == ET - 1))
                o_sb = outp.tile([128, D], F32, tag="o_sb")
                nc.vector.tensor_copy(o_sb[:], ps_out[:])
                nc.sync.dma_start(out=out_d[st * 128 : (st + 1) * 128, :], in_=o_sb[:])

    nc.compile()
    return nc


_CACHED = {}


def run(inputs: dict, trace: bool = False, **spmd_kwargs):
    if "nc" not in _CACHED:
        _CACHED["nc"] = build_bass()
    nc = _CACHED["nc"]

    q = np.ascontiguousarray(np.asarray(inputs["q"], dtype=np.float32))
    k = np.ascontiguousarray(np.asarray(inputs["k"], dtype=np.float32))
    v = np.ascontiguousarray(np.asarray(inputs["v"], dtype=np.float32))
    mask = np.asarray(inputs["mask"], dtype=np.float32)
    bias = np.ascontiguousarray(
        np.asarray(inputs["bias"], dtype=np.float32).reshape(H, S, S))
    ws = {n: np.ascontiguousarray(np.asarray(inputs[n], dtype=np.float32))
          for n in ("Wq", "Wk", "Wv", "Wg", "bg", "Wo")}

    B = q.shape[0]
    in_maps = []
    for b in range(B):
        # additive mask is per-(batch, k); fold into the replicated bias copy
        if np.any(mask[b]):
            bias_b = np.ascontiguousarray(bias + mask[b].reshape(1, 1, S))
        else:
            bias_b = bias
        in_maps.append({
            "q": np.ascontiguousarray(q[b]),
            "k": np.ascontiguousarray(k[b]),
            "v": np.ascontiguousarray(v[b]),
            "bias": bias_b,
            **ws,
        })
    res = run_bass_kernel_spmd(nc, in_maps, list(range(NCORES)),
                               trace=trace, **spmd_kwargs)
    out = np.stack([res.results[i]["out"] for i in range(NCORES)], axis=0)
    return out, res


def kernel(**inputs) -> np.ndarray:
    out, _ = run(inputs)
    return out.astype(np.float32)
